# revision 42
# baseline (speedup 1.0000x reference)
"""Trainium2 Bass kernel for nn_Aligner: dual transformer encoder + pairwise
log-softmax alignment. Data-parallel over batch B=8 across 8 NeuronCores
(one batch element per core); encoder weights + embedding tables replicated
per core; embedding lookup on-device via indirect DMA.

Numerics: fp8(e4m3) DoubleRow matmuls (2x PE rate) with fp32 PSUM, fp16
QK^T scores, fp32 stats/softmax. Static power-of-2 scaling keeps fp8
operands in the normal range: activations x16, weights x32, PSUM x512;
all compensations are folded into existing post-PSUM ops host-side.

The PE executes matmuls in order, so a matmul that waits on a LayerNorm's
DVE statistics chain stalls everything behind it. The text and unit
encoders are therefore emitted as interleaved chunks: while one encoder's
LN chain runs on DVE/ACT, the other encoder's matmul stream keeps the PE
busy.
"""

import numpy as np
import ml_dtypes
from contextlib import ExitStack

import concourse.bass as bass
import concourse.tile as tile
from concourse import bacc, mybir
from concourse.bass_utils import run_bass_kernel_spmd
from concourse.masks import make_identity

# model constants (hardcoded per problem spec)
D, H, DH, FFD, L = 1024, 8, 128, 2048, 3
KD = D // 128          # 8 feature tiles
KF = FFD // 128        # 16 ff tiles
T, U, B = 256, 1024, 8
LN_EPS, TEMP, NEG = 1e-5, 5e-4, -1e9
SCALE = float(1.0 / np.sqrt(DH).astype(np.float32))

SA = 16.0              # activation fp8 scale
SW = 32.0              # weight fp8 scale
SP = SA * SW           # psum scale after one fp8 matmul (512)
CS = SP / SA           # ln sd carries CS*sqrt(var+eps)

F8, F16, F32, I32 = (mybir.dt.float8e4, mybir.dt.float16,
                     mybir.dt.float32, mybir.dt.int32)
AOP = mybir.AluOpType
AF = mybir.ActivationFunctionType
DRM = mybir.MatmulPerfMode.DoubleRow
NP8 = ml_dtypes.float8_e4m3

TRACE = False
TRACE_DIR = None
LAST_RESULTS = None
_CACHE = {}


def _nblocks(S):
    return [(n0, min(512, S - n0)) for n0 in range(0, S, 512)]


class _Ctx:
    pass


def _encoder_chunks(g, pfx, S, V, out_into=None):
    """Generator emitting one transformer encoder in chunks (yield between
    chunks so two encoders can interleave). Final per-block activations
    [128, KD, NB] (fp8, x16) are stored in g.result[pfx]."""
    nc, p = g.nc, g.pools
    NT = S // 128
    dram = g.dram
    blocks = _nblocks(S)
    NBL = len(blocks)

    idx = p["cst"].tile([128, NT], I32, tag=f"idx_{pfx}")
    nc.sync.dma_start(idx[:], dram[f"{pfx}_tok"].rearrange("(t p) -> p t", p=128))
    kb = p["cst"].tile([128, NT], F32, tag=f"kb_{pfx}")
    nc.sync.dma_start(kb[:], dram[f"{pfx}_kb"].rearrange("(t p) -> p t", p=128))

    def new_xblocks(name):
        return [p["stream8"].tile([128, KD, NB], F8, tag=f"s8_{pfx}_{bi}", name=name)
                for bi, (n0, NB) in enumerate(blocks)]

    # --- chunk: embedding gather (x16 fp16 table) + PE transpose ---
    xTb = new_xblocks("xT")
    for st in range(NT):
        bi, lo = st // 4, (st % 4) * 128
        x0 = p["x0"].tile([128, D], F16, tag="x0")
        nc.gpsimd.indirect_dma_start(
            out=x0[:], out_offset=None, in_=dram[f"{pfx}_emb"][:],
            in_offset=bass.IndirectOffsetOnAxis(ap=idx[:, st:st + 1], axis=0))
        tp = p["psO"].tile([128, KD * 128], F16, tag="psO")
        for kd in range(KD):
            nc.tensor.transpose(tp[:, kd * 128:(kd + 1) * 128],
                                x0[:, kd * 128:(kd + 1) * 128], g.ident[:])
        nc.vector.tensor_copy(xTb[bi][:, :, lo:lo + 128],
                              tp[:].rearrange("p (k c) -> p k c", k=KD))
    yield

    def ln_squares(yb):
        """M-chunk tail: per-block squares t=(y/1024)*y on DVE (no PE dep)."""
        sqs = []
        for bi, (n0, NB) in enumerate(blocks):
            tbig = p["tbig"].tile([128, KD, 512], F16, tag="tbig")
            for kd in range(0, KD, 2):
                nc.scalar.activation(tbig[:, kd:kd + 2, :NB],
                                     yb[bi][:, kd:kd + 2, :NB], AF.Square,
                                     scale=1.0 / 32.0)
            sqs.append(tbig)
        return sqs

    def ln_rows(yb, sqs):
        """W-chunk: ss/sq reduction matmuls + row chain -> (a_t, c_t) rows."""
        out = []
        sss = []
        for bi, (n0, NB) in enumerate(blocks):
            ss = p["psRow"].tile([1, NB], F32, tag="psRow")
            for kd in range(KD):
                nc.tensor.matmul(ss[:], g.ones_c16[:], yb[bi][:, kd, :NB],
                                 start=(kd == 0), stop=(kd == KD - 1))
            sss.append(ss)
        for bi, (n0, NB) in enumerate(blocks):
            ss = sss[bi]
            msb = p["row"].tile([1, 512], F32, tag="row")
            nc.vector.tensor_scalar(msb[:, :NB], ss[:], 1.0 / (SP * D), 0.0,
                                    op0=AOP.mult, op1=AOP.add)
            m2 = p["row"].tile([1, 512], F32, tag="row")
            nc.vector.tensor_tensor(m2[:, :NB], msb[:, :NB], msb[:, :NB],
                                    op=AOP.mult)
            sq = p["psRow"].tile([1, NB], F32, tag="psRow")
            for kd in range(KD):
                nc.tensor.matmul(sq[:], g.ones_c16[:], sqs[bi][:, kd, :NB],
                                 start=(kd == 0), stop=(kd == KD - 1))
            var = p["row"].tile([1, 512], F32, tag="row")
            nc.vector.scalar_tensor_tensor(var[:, :NB], in0=sq[:],
                                           scalar=1024.0 / (SP * SP * D),
                                           in1=m2[:, :NB], op0=AOP.mult,
                                           op1=AOP.subtract)
            sd = p["row"].tile([1, 512], F32, tag="row")
            nc.scalar.activation(sd[:, :NB], var[:, :NB], AF.Sqrt,
                                 scale=CS * CS, bias=g.eps2[:, :1])
            a32 = p["row"].tile([1, 512], F32, tag="row")
            nc.vector.reciprocal_approx_fast(a32[:, :NB], sd[:, :NB])
            a_t = p["row16"].tile([1, 512], F16, tag=f"ra_{pfx}")
            nc.vector.tensor_copy(a_t[:, :NB], a32[:, :NB])
            c_t = p["row16"].tile([1, 512], F16, tag=f"rc_{pfx}")
            nc.vector.scalar_tensor_tensor(c_t[:, :NB], in0=msb[:, :NB],
                                           scalar=-SP, in1=a32[:, :NB],
                                           op0=AOP.mult, op1=AOP.mult)
            out.append((a_t, c_t))
        return out

    def ln_write(stats, yb, s_t, b_t, dst=None):
        """A/C broadcast matmuls + xn = y*A + C (fp8 x16)."""
        xnb = new_xblocks("xn") if dst is None else dst
        ACs = {}
        border = list(enumerate(blocks))[::-1]
        for bi, (n0, NB) in border:
            a_t, c_t = stats[bi]
            A = p["psMM"].tile([128, 512], F32, tag="psMM")
            nc.tensor.matmul(A[:, :NB], g.ones_rw16[:, :128], a_t[:, :NB],
                             start=True, stop=True)
            C = p["psO"].tile([128, 512], F32, tag="psO", name="Cb")
            nc.tensor.matmul(C[:, :NB], g.ones_rw16[:, :128], c_t[:, :NB],
                             start=True, stop=True)
            ACs[bi] = (A, C)
        for bi, (n0, NB) in border:
            A, C = ACs[bi]
            y = yb[bi]
            A16 = p["rsb"].tile([128, 512], F16, tag="rsb")
            nc.vector.tensor_copy(A16[:, :NB], A[:, :NB])
            C16 = p["rsb"].tile([128, 512], F16, tag="rsb")
            nc.vector.tensor_copy(C16[:, :NB], C[:, :NB])
            # all-f16 SBUF operands -> DVE 2x mode; fp8 quantize on ACT
            t1 = p["tbig"].tile([128, KD, 512], F16, tag="tbig")
            nc.vector.tensor_tensor(t1[:, :, :NB], y[:, :, :NB],
                                    A16[:, None, :NB].broadcast_to([128, KD, NB]),
                                    op=AOP.mult)
            nc.vector.tensor_tensor(t1[:, :, :NB], t1[:, :, :NB],
                                    C16[:, None, :NB].broadcast_to([128, KD, NB]),
                                    op=AOP.add)
            if s_t is None:
                nc.scalar.activation(xnb[bi][:, :, :NB], t1[:, :, :NB], AF.Identity)
            else:
                for kd in range(KD):
                    nc.scalar.activation(xnb[bi][:, kd, :NB], t1[:, kd, :NB],
                                         AF.Identity, scale=s_t[:, kd:kd + 1],
                                         bias=b_t[:, kd:kd + 1])
        return xnb

    for l in range(L):
        w_r = dram[f"{pfx}w_{l}"]
        wv_r = dram[f"{pfx}wv_{l}"]
        ow_r = dram[f"{pfx}ow_{l}"]
        f1_r = dram[f"{pfx}f1_{l}"]
        f2_r = dram[f"{pfx}f2_{l}"]
        ipb_t = p["pp"].tile([128, 24], F32, tag="ipb")
        nc.sync.dma_start(ipb_t[:], dram[f"{pfx}ipb_{l}"].rearrange("(j p) -> p j", p=128))
        f1b_t = p["pp"].tile([128, KF], F32, tag="f1b")
        nc.sync.dma_start(f1b_t[:], dram[f"{pfx}f1b_{l}"].rearrange("(j p) -> p j", p=128))
        ls1_t = p["pp"].tile([128, KD], F32, tag="ls1")
        nc.sync.dma_start(ls1_t[:], dram[f"{pfx}ls1_{l}"].rearrange("(j p) -> p j", p=128))
        ps_t = p["pp"].tile([128, KD], F32, tag="ps")
        nc.sync.dma_start(ps_t[:], dram[f"{pfx}ps_{l}"].rearrange("(j p) -> p j", p=128))
        rb1 = p["row16"].tile([1, D], F16, tag="rb1")
        nc.sync.dma_start(rb1[:], dram[f"{pfx}rb1_{l}"][None, :])
        rb2 = p["row16"].tile([1, D], F16, tag="rb2")
        nc.sync.dma_start(rb2[:], dram[f"{pfx}rb2_{l}"][None, :])
        if l == L - 1:
            ls2_t = p["pp"].tile([128, KD], F32, tag="ls2")
            nc.sync.dma_start(ls2_t[:], dram[f"{pfx}ls2_{l}"].rearrange("(j p) -> p j", p=128))
            lb2_t = p["pp"].tile([128, KD], F32, tag="lb2")
            nc.sync.dma_start(lb2_t[:], dram[f"{pfx}lb2_{l}"].rearrange("(j p) -> p j", p=128))

        # ==== chunk m1: V projection, attention heads, O projection + y ====
        v4 = p["v"].tile([128, NT, H, DH], F8, tag="v")
        wv = p["wv"].tile([128, KD, D], F8, tag="wv")
        nc.sync.dma_start(wv[:], wv_r[:])
        vb32 = p["row"].tile([1, D], F32, tag="rowb", bufs=2)
        nc.sync.dma_start(vb32[:], dram[f"{pfx}ipb_{l}"][None, 2 * D:3 * D])
        vb16 = p["row16"].tile([1, D], F16, tag="row16", bufs=2)
        nc.vector.tensor_copy(vb16[:], vb32[:])
        for st in range(NT):
            bi, lo = st // 4, (st % 4) * 128
            ps0 = p["psMM"].tile([128, 512], F32, tag="psMM")
            ps1 = p["psMM"].tile([128, 512], F32, tag="psMM")
            for kd in range(0, KD, 2):
                lh = xTb[bi][:, kd:kd + 2, lo:lo + 128]
                nc.tensor.matmul(ps0[:], lh, wv[:, kd:kd + 2, 0:512],
                                 start=(kd == 0), stop=False, perf_mode=DRM)
                nc.tensor.matmul(ps1[:], lh, wv[:, kd:kd + 2, 512:1024],
                                 start=(kd == 0), stop=False, perf_mode=DRM)
            nc.tensor.matmul(ps0[:], g.ones_r16[:], vb16[:, 0:512],
                             start=False, stop=True, skip_group_check=True)
            nc.tensor.matmul(ps1[:], g.ones_r16[:], vb16[:, 512:1024],
                             start=False, stop=True, skip_group_check=True)
            nc.vector.tensor_scalar(v4[:, st, 0:4, :],
                                    ps0[:].rearrange("p (a b) -> p a b", a=4),
                                    1.0 / SW, 0.0, op0=AOP.mult, op1=AOP.add)
            nc.vector.tensor_scalar(v4[:, st, 4:8, :],
                                    ps1[:].rearrange("p (a b) -> p a b", a=4),
                                    1.0 / SW, 0.0, op0=AOP.mult, op1=AOP.add)

        oTb = [p["oT"].tile([128, H, NB], F8, tag=f"oT_{bi}", name="oT")
               for bi, (n0, NB) in enumerate(blocks)]
        for h in range(H):
            qk = p["qk"].tile([128, 2, S], F16, tag="qk")
            for part in range(2):
                wq = p["wcol"].tile([128, KD, 128], F8, tag="wcol")
                nc.sync.dma_start(wq[:], w_r[part * 8 + h])
                pss = [p["psMM"].tile([128, 512], F32, tag="psMM", name="psp")
                       for _ in blocks]
                for kd in range(0, KD, 2):
                    for ps, bi in zip(pss, range(NBL)):
                        NB = blocks[bi][1]
                        nc.tensor.matmul(ps[:, :NB], wq[:, kd:kd + 2, :],
                                         xTb[bi][:, kd:kd + 2, :NB],
                                         start=(kd == 0), stop=(kd == KD - 2),
                                         perf_mode=DRM)
                for ps, (n0, NB) in zip(pss, blocks):
                    nc.vector.tensor_scalar(qk[:, part, n0:n0 + NB], ps[:, :NB],
                                            ipb_t[:, part * 8 + h:part * 8 + h + 1],
                                            0.0, op0=AOP.add, op1=AOP.add)
            attnb = [p["attn"].tile([128, NT, NB], F8, tag=f"attn_{bi}", name="attn")
                     for bi, (n0, NB) in enumerate(blocks)]
            for kt in range(NT):
                pss = [p["psMM"].tile([128, 512], F32, tag="psMM", name="psp")
                       for _ in blocks]
                for ps, (n0, NB) in zip(pss, blocks):
                    nc.tensor.matmul(ps[:, :NB], qk[:, 1, kt * 128:(kt + 1) * 128],
                                     qk[:, 0, n0:n0 + NB], start=True, stop=True)
                for ps, bi, (n0, NB) in zip(pss, range(NBL), blocks):
                    nc.scalar.activation(attnb[bi][:, kt, :NB], ps[:, :NB], AF.Exp,
                                         scale=SCALE / (SP * SP), bias=kb[:, kt:kt + 1])
            srows = []
            for bi, (n0, NB) in enumerate(blocks):
                rsum = p["psRow"].tile([1, NB], F32, tag="psRow")
                for kt in range(0, NT, 2):
                    nc.tensor.matmul(rsum[:], g.ones8[:, :, 0:1],
                                     attnb[bi][:, kt:kt + 2, :NB],
                                     start=(kt == 0), stop=(kt == NT - 2),
                                     perf_mode=DRM)
                srow32 = p["row"].tile([1, 512], F32, tag="srow32", bufs=2)
                nc.vector.reciprocal_approx_fast(srow32[:, :NB], rsum[:])
                srow = p["row16"].tile([1, 512], F16, tag="srow")
                nc.vector.tensor_copy(srow[:, :NB], srow32[:, :NB])
                srows.append(srow)
            for bi, (n0, NB) in enumerate(blocks):
                po = p["psO"].tile([128, 512], F32, tag="psO", name="po")
                for kt in range(0, NT, 2):
                    nc.tensor.matmul(po[:, :NB], v4[:, kt:kt + 2, h, :],
                                     attnb[bi][:, kt:kt + 2, :NB],
                                     start=(kt == 0), stop=(kt == NT - 2),
                                     perf_mode=DRM)
                rs = p["psB"].tile([128, 512], F32, tag="psB")
                nc.tensor.matmul(rs[:, :NB], g.ones_rw16[:, :128],
                                 srows[bi][:, :NB], start=True, stop=True)
                rsb = p["rsb"].tile([128, 512], F16, tag="rsb")
                nc.vector.tensor_copy(rsb[:, :NB], rs[:, :NB])
                nc.vector.tensor_tensor(oTb[bi][:, h, :NB], po[:, :NB],
                                        rsb[:, :NB], op=AOP.mult)

        yb = [p["streamY"].tile([128, KD, NB], F16, tag=f"y_{pfx}_{bi}", name="y")
              for bi, (n0, NB) in enumerate(blocks)]
        for m in range(KD):
            wo = p["wcol"].tile([128, KD, 128], F8, tag="wcol")
            nc.sync.dma_start(wo[:], ow_r[m])
            pss = [p["psMM"].tile([128, 512], F32, tag="psMM", name="psp")
                   for _ in blocks]
            for kh in range(0, H, 2):
                for ps, bi in zip(pss, range(NBL)):
                    NB = blocks[bi][1]
                    nc.tensor.matmul(ps[:, :NB], wo[:, kh:kh + 2, :],
                                     oTb[bi][:, kh:kh + 2, :NB],
                                     start=(kh == 0), stop=False, perf_mode=DRM)
            for ps, bi, (n0, NB) in zip(pss, range(NBL), blocks):
                nc.tensor.matmul(ps[:, :NB], rb1[:, m * 128:(m + 1) * 128],
                                 g.ones_rw16[:, :NB], start=False, stop=True,
                                 skip_group_check=True)
                nc.vector.scalar_tensor_tensor(yb[bi][:, m, :NB],
                                               in0=xTb[bi][:, m, :NB],
                                               scalar=ps_t[:, m:m + 1],
                                               in1=ps[:, :NB],
                                               op0=AOP.mult, op1=AOP.add)
        sq1 = ln_squares(yb)
        yield

        # ==== chunk W1: LN1 reductions + xln write ====
        st1 = ln_rows(yb, sq1)
        xlnb = ln_write(st1, yb, None, None)
        yield

        # ==== chunk M2: feed-forward + y2 ====
        y2b = [p["streamY"].tile([128, KD, NB], F16, tag=f"y_{pfx}_{bi}", name="y2")
               for bi, (n0, NB) in enumerate(blocks)]
        ffbs = [p["big"].tile([128, KF, NB], F8, tag=f"big_{bi}", name="ffb")
                for bi, (n0, NB) in enumerate(blocks)]
        for m in range(KF):
            wf = p["wcol"].tile([128, KD, 128], F8, tag="wcol")
            nc.sync.dma_start(wf[:], f1_r[m])
            pss = [p["psMM"].tile([128, 512], F32, tag="psMM", name="psp")
                   for _ in blocks]
            for kd in range(0, KD, 2):
                for ps, bi in zip(pss, range(NBL)):
                    NB = blocks[bi][1]
                    nc.tensor.matmul(ps[:, :NB], wf[:, kd:kd + 2, :],
                                     xlnb[bi][:, kd:kd + 2, :NB],
                                     start=(kd == 0), stop=(kd == KD - 2),
                                     perf_mode=DRM)
            for ps, bi, (n0, NB) in zip(pss, range(NBL), blocks):
                nc.scalar.activation(ffbs[bi][:, m, :NB], ps[:, :NB], AF.Relu,
                                     scale=SA / SP, bias=f1b_t[:, m:m + 1])
        for m2 in range(KD):
            wf2 = p["wcol2"].tile([128, KF, 128], F8, tag="wcol2")
            nc.sync.dma_start(wf2[:], f2_r[m2])
            pss = [p["psMM"].tile([128, 512], F32, tag="psMM", name="psp")
                   for _ in blocks]
            for kf in range(0, KF, 2):
                for ps, bi in zip(pss, range(NBL)):
                    NB = blocks[bi][1]
                    nc.tensor.matmul(ps[:, :NB], wf2[:, kf:kf + 2, :],
                                     ffbs[bi][:, kf:kf + 2, :NB],
                                     start=(kf == 0), stop=False, perf_mode=DRM)
            for ps, bi, (n0, NB) in zip(pss, range(NBL), blocks):
                nc.tensor.matmul(ps[:, :NB], rb2[:, m2 * 128:(m2 + 1) * 128],
                                 g.ones_rw16[:, :NB], start=False, stop=True,
                                 skip_group_check=True)
                nc.vector.scalar_tensor_tensor(y2b[bi][:, m2, :NB],
                                               in0=xlnb[bi][:, m2, :NB],
                                               scalar=ls1_t[:, m2:m2 + 1],
                                               in1=ps[:, :NB],
                                               op0=AOP.mult, op1=AOP.add)
        sq2 = ln_squares(y2b)
        yield

        # ==== chunk W2: LN2 reductions + write ====
        st2 = ln_rows(y2b, sq2)
        if l == L - 1:
            xTb = ln_write(st2, y2b, ls2_t, lb2_t, dst=out_into)
        else:
            xTb = ln_write(st2, y2b, None, None)
        if l < L - 1:
            yield
    g.result[pfx] = xTb


def _build(Vt, Vu):
    nc = bacc.Bacc("TRN2", target_bir_lowering=False, debug=False, num_devices=B)
    dram = {}
    dram["t_tok"] = nc.dram_tensor("t_tok", [T], I32, kind="ExternalInput").ap()
    dram["u_tok"] = nc.dram_tensor("u_tok", [U], I32, kind="ExternalInput").ap()
    dram["t_emb"] = nc.dram_tensor("t_emb", [Vt, D], F16, kind="ExternalInput").ap()
    dram["u_emb"] = nc.dram_tensor("u_emb", [Vu, D], F16, kind="ExternalInput").ap()
    dram["t_kb"] = nc.dram_tensor("t_kb", [T], F32, kind="ExternalInput").ap()
    dram["u_kb"] = nc.dram_tensor("u_kb", [U], F32, kind="ExternalInput").ap()
    dram["tmask"] = nc.dram_tensor("tmask", [T], F32, kind="ExternalInput").ap()
    for e in ("t", "u"):
        for l in range(L):
            for name, shape, dt in [
                (f"{e}w_{l}", [16, 128, KD, 128], F8),
                (f"{e}wv_{l}", [128, KD, D], F8),
                (f"{e}ipb_{l}", [3 * D], F32),
                (f"{e}ow_{l}", [8, 128, KD, 128], F8),
                (f"{e}f1_{l}", [16, 128, KD, 128], F8), (f"{e}f1b_{l}", [FFD], F32),
                (f"{e}f2_{l}", [8, 128, KF, 128], F8),
                (f"{e}ls1_{l}", [D], F32), (f"{e}ps_{l}", [D], F32),
                (f"{e}rb1_{l}", [D], F16), (f"{e}rb2_{l}", [D], F16),
                (f"{e}ls2_{l}", [D], F32), (f"{e}lb2_{l}", [D], F32),
            ]:
                dram[name] = nc.dram_tensor(name, shape, dt, kind="ExternalInput").ap()
    out = nc.dram_tensor("out", [U, T], F32, kind="ExternalOutput").ap()

    g = _Ctx()
    g.nc = nc
    g.dram = dram
    g.result = {}

    with tile.TileContext(nc) as tc:
        with ExitStack() as es:
            p = {}
            p["cst"] = es.enter_context(tc.tile_pool(name="cst", bufs=1))
            p["stream8"] = es.enter_context(tc.tile_pool(name="stream8", bufs=2))
            p["streamY"] = es.enter_context(tc.tile_pool(name="streamY", bufs=1))
            p["tf"] = es.enter_context(tc.tile_pool(name="tf", bufs=1))
            p["qk"] = es.enter_context(tc.tile_pool(name="qk", bufs=2))
            p["v"] = es.enter_context(tc.tile_pool(name="v", bufs=2))
            p["big"] = es.enter_context(tc.tile_pool(name="big", bufs=1))
            p["attn"] = es.enter_context(tc.tile_pool(name="attn", bufs=2))
            p["oT"] = es.enter_context(tc.tile_pool(name="oT", bufs=1))
            p["tbig"] = es.enter_context(tc.tile_pool(name="tbig", bufs=3))
            p["tmp32"] = es.enter_context(tc.tile_pool(name="tmp32", bufs=2))
            p["x0"] = es.enter_context(tc.tile_pool(name="x0", bufs=2))
            p["wcol"] = es.enter_context(tc.tile_pool(name="wcol", bufs=4))
            p["wcol2"] = es.enter_context(tc.tile_pool(name="wcol2", bufs=2))
            p["wv"] = es.enter_context(tc.tile_pool(name="wv", bufs=1))
            p["row"] = es.enter_context(tc.tile_pool(name="row", bufs=5))
            p["row16"] = es.enter_context(tc.tile_pool(name="row16", bufs=2))
            p["rsb"] = es.enter_context(tc.tile_pool(name="rsb", bufs=2))
            p["pp"] = es.enter_context(tc.tile_pool(name="pp", bufs=2))
            p["psMM"] = es.enter_context(tc.tile_pool(name="psMM", bufs=3, space="PSUM"))
            p["psO"] = es.enter_context(tc.tile_pool(name="psO", bufs=2, space="PSUM"))
            p["psRow"] = es.enter_context(tc.tile_pool(name="psRow", bufs=2, space="PSUM"))
            p["psB"] = es.enter_context(tc.tile_pool(name="psB", bufs=1, space="PSUM"))
            g.pools = p

            g.ident = p["cst"].tile([128, 128], F16, tag="ident")
            make_identity(nc, g.ident[:])
            g.ones_c16 = p["cst"].tile([128, 1], F16, tag="oc16")
            nc.vector.memset(g.ones_c16[:], 1.0)
            g.ones_r16 = p["cst"].tile([1, 128], F16, tag="or16")
            nc.vector.memset(g.ones_r16[:], 1.0)
            g.ones_rw16 = p["cst"].tile([1, 512], F16, tag="orw16")
            nc.vector.memset(g.ones_rw16[:], 1.0)
            g.ones_r32 = p["cst"].tile([1, 128], F32, tag="or32")
            nc.vector.memset(g.ones_r32[:], 1.0)
            g.ones8 = p["cst"].tile([128, 2, 16], F8, tag="o8")
            nc.vector.memset(g.ones8[:], 1.0)
            g.eps2 = p["cst"].tile([1, 1], F32, tag="eps2")
            nc.vector.memset(g.eps2[:], LN_EPS * CS * CS)

            tf_dst = [p["tf"].tile([128, KD, 256], F8, tag="tf", name="tf")]
            gu = _encoder_chunks(g, "u", U, Vu)
            gt = _encoder_chunks(g, "t", T, Vt, out_into=tf_dst)
            def emit_tn():
                # tn[t] = sum_d tf^2 (x256) ; g_row = -0.5*tnp + tmask (x256)
                tfT = g.result["t"][0]
                tmask_t = p["row"].tile([1, 512], F32, tag="row")
                nc.sync.dma_start(tmask_t[:, :T], dram["tmask"][None, :])
                tnp = p["psRow"].tile([1, T], F32, tag="psRow")
                for kd in range(KD):
                    tsq = p["rsb"].tile([128, 512], F16, tag="rsb")
                    nc.vector.tensor_tensor(tsq[:, :T], tfT[:, kd, :T],
                                            tfT[:, kd, :T], op=AOP.mult)
                    nc.tensor.matmul(tnp[:], g.ones_c16[:], tsq[:, :T],
                                     start=(kd == 0), stop=(kd == KD - 1))
                grow = p["cst"].tile([1, T], F32, tag="grow")
                nc.vector.scalar_tensor_tensor(grow[:], in0=tnp[:], scalar=-0.5,
                                               in1=tmask_t[:, :T], op0=AOP.mult,
                                               op1=AOP.add)
                return grow

            grow = None
            du = dt_ = False
            while not (du and dt_):
                if not dt_:
                    try:
                        next(gt)
                    except StopIteration:
                        dt_ = True
                        grow = emit_tn()
                if not du:
                    try:
                        next(gu)
                    except StopIteration:
                        du = True
            ufb = g.result["u"]
            tfT = g.result["t"][0]

            # final: per u-tile cross matmul (fp8 DR) + log-softmax over t
            for ut in range(U // 128):
                bi, lo = ut // 4, (ut % 4) * 128
                ps = p["psO"].tile([128, 512], F32, tag="psO", name="fps")
                for kd in range(0, KD, 2):
                    nc.tensor.matmul(ps[:, :T], ufb[bi][:, kd:kd + 2, lo:lo + 128],
                                     tfT[:, kd:kd + 2, :], start=(kd == 0), stop=False,
                                     perf_mode=DRM)
                nc.tensor.matmul(ps[:, :T], g.ones_r32[:], grow[:], start=False,
                                 stop=True, skip_group_check=True)
                e32 = p["tmp32"].tile([128, 512], F32, tag="tmp32")
                acc = p["pp"].tile([128, 1], F32, tag="acc")
                nc.scalar.activation(e32[:, :T], ps[:, :T], AF.Exp,
                                     scale=2.0 * TEMP / (SA * SA), accum_out=acc[:, :1])
                lse = p["pp"].tile([128, 1], F32, tag="lse")
                nc.scalar.activation(lse[:, :1], acc[:, :1], AF.Ln)
                ot = p["tmp32"].tile([128, 512], F32, tag="tmp32")
                nc.vector.tensor_scalar(ot[:, :T], ps[:, :T], 2.0 * TEMP / (SA * SA),
                                        lse[:, :1], op0=AOP.mult, op1=AOP.subtract)
                nc.sync.dma_start(out[ut * 128:(ut + 1) * 128, :], ot[:, :T])

    nc.compile()
    return nc


def _tile_w(wT):
    """[Din, C] -> [C//128, 128, Din//128, 128]: each output [cb] block is the
    contiguous SBUF image of one stationary-column load (p, k, c)."""
    Din, C = wT.shape
    t = wT.reshape(Din // 128, 128, C // 128, 128)   # (k, p, cb, c)
    return np.ascontiguousarray(t.transpose(2, 1, 0, 3))  # (cb, p, k, c)


def _q8(a):
    """fp32 -> TRN fp8e4 (e4m3, max +-240) with clipping."""
    return np.clip(a, -240.0, 240.0).astype(NP8)


def _prep_in_maps(inputs):
    f32 = np.float32
    tok_t = np.ascontiguousarray(np.asarray(inputs["text_tokens"]).astype(np.int32))
    tok_u = np.ascontiguousarray(np.asarray(inputs["unit_tokens"]).astype(np.int32))
    Vt = inputs["t_emb"].shape[0]
    Vu = inputs["u_emb"].shape[0]

    shared = {
        "t_emb": np.ascontiguousarray(
            (np.asarray(inputs["t_emb"], f32) * SA).astype(np.float16)),
        "u_emb": np.ascontiguousarray(
            (np.asarray(inputs["u_emb"], f32) * SA).astype(np.float16)),
    }
    for e in ("t", "u"):
        pf = e + "_"
        # LayerNorm affine folding: each non-final LN's scale/bias is folded
        # into the next consumer's weights/biases (exact when scale=1, bias=0).
        prev_s = np.ones(D, f32)
        prev_b = np.zeros(D, f32)
        for l in range(L):
            ipw = np.asarray(inputs[pf + "ipw"][l], f32)
            ipb = np.asarray(inputs[pf + "ipb"][l], f32)
            ow = np.asarray(inputs[pf + "ow"][l], f32)
            ob = np.asarray(inputs[pf + "ob"][l], f32)
            f1w = np.asarray(inputs[pf + "f1w"][l], f32)
            f1b = np.asarray(inputs[pf + "f1b"][l], f32)
            f2w = np.asarray(inputs[pf + "f2w"][l], f32)
            f2b = np.asarray(inputs[pf + "f2b"][l], f32)
            l1s = np.asarray(inputs[pf + "l1s"][l], f32)
            l1b = np.asarray(inputs[pf + "l1b"][l], f32)
            wT = (ipw * prev_s[None, :]).T * SW      # [Din, 3D], fp8 weight scale
            shared[f"{e}w_{l}"] = _q8(_tile_w(wT[:, :2 * D]))
            shared[f"{e}wv_{l}"] = _q8(np.ascontiguousarray(
                wT[:, 2 * D:].reshape(KD, 128, D).transpose(1, 0, 2)))
            shared[f"{e}ipb_{l}"] = np.ascontiguousarray((ipb + ipw @ prev_b) * SP)
            shared[f"{e}ow_{l}"] = _q8(_tile_w(ow.T * SW))
            shared[f"{e}ps_{l}"] = np.ascontiguousarray(prev_s * SW)
            shared[f"{e}rb1_{l}"] = np.ascontiguousarray(
                ((ob + prev_b) * SP).astype(np.float16))
            shared[f"{e}f1_{l}"] = _q8(_tile_w((f1w * l1s[None, :]).T * SW))
            shared[f"{e}f1b_{l}"] = np.ascontiguousarray((f1b + f1w @ l1b) * SA)
            shared[f"{e}f2_{l}"] = _q8(_tile_w(f2w.T * SW))
            shared[f"{e}rb2_{l}"] = np.ascontiguousarray(
                ((f2b + l1b) * SP).astype(np.float16))
            shared[f"{e}ls1_{l}"] = np.ascontiguousarray(l1s * SW)
            shared[f"{e}ls2_{l}"] = np.ascontiguousarray(np.asarray(inputs[pf + "l2s"][l], f32))
            shared[f"{e}lb2_{l}"] = np.ascontiguousarray(
                np.asarray(inputs[pf + "l2b"][l], f32) * SA)
            prev_s = np.asarray(inputs[pf + "l2s"][l], f32)
            prev_b = np.asarray(inputs[pf + "l2b"][l], f32)

    in_maps = []
    for c in range(B):
        tpad = tok_t[c] == Vt - 1
        upad = tok_u[c] == Vu - 1
        m = dict(shared)
        m["t_tok"] = tok_t[c]
        m["u_tok"] = tok_u[c]
        m["t_kb"] = np.where(tpad, np.float32(NEG), np.float32(0.0)).astype(f32)
        m["u_kb"] = np.where(upad, np.float32(NEG), np.float32(0.0)).astype(f32)
        m["tmask"] = np.where(tpad, np.float32(NEG * SA * SA / (2.0 * TEMP)),
                              np.float32(0.0)).astype(f32)
        in_maps.append(m)
    return in_maps, tok_u, Vu


def kernel(**inputs):
    global LAST_RESULTS
    Vt = inputs["t_emb"].shape[0]
    Vu = inputs["u_emb"].shape[0]
    key = (Vt, Vu)
    if key not in _CACHE:
        _CACHE[key] = _build(Vt, Vu)
    nc = _CACHE[key]

    in_maps, tok_u, Vu_ = _prep_in_maps(inputs)
    kw = {}
    if TRACE:
        kw = dict(trace=True, tmpdir=TRACE_DIR)
    br = run_bass_kernel_spmd(nc, in_maps, list(range(B)), **kw)
    LAST_RESULTS = br

    out = np.stack([br.results[c]["out"] for c in range(B)], axis=0)
    # padded unit rows: reference gives uniform -log(T) rows (never triggers
    # with the spec's token distribution, but exact when it does)
    for c in range(B):
        upad = tok_u[c] == Vu_ - 1
        if upad.any():
            out[c, upad, :] = -np.log(np.float32(T)).astype(np.float32)
    return out
